# revision 8
# baseline (speedup 1.0000x reference)
"""Trainium2 Bass kernel for nn_CumulativeFlattenedLinear (segment_reduce).

Per window of S=64 timesteps: per-timestep C->O projection (weights zero for
the first n_discard steps) + causal cumsum within the window, plus bias.

Strategy (data-parallel over batch, 1 batch element per core):
  - Host marshals x to fp16, drops the n_discard zero-weight positions of
    every window (25% of the data), pre-transposed as x^T[(c,v'), ...] so the
    device reads matmul stationary slabs directly.
  - Per window: DU triangular "intra" matmuls produce within-block causal
    cumsums in PSUM; packed "pre" matmuls accumulate block-prefix totals into
    a 128-col pre region in its own PSUM bank (first pre matmul covers all
    DU targets, with a zero block so du=0 reads exact 0).  Intra and pre
    matmuls are interleaved so each x-block's stationary weights are loaded
    once.
  - Eviction: one DVE tensor-tensor ADD per window reads intra + broadcast
    pre straight from PSUM and writes (o, s)-major fp16 output tiles.
  - Bias is applied host-side (discard positions are bias-only, also filled
    host-side), so the device does no bias work.
  - PE warm-up matmuls keep the HAM clock busy until pair 0's data lands;
    loads are ordered so pair 0's chunk is first on the wire.
"""
import numpy as np

import concourse.bass as bass
import concourse.tile as tile
from concourse import bacc, mybir
from concourse.bass_utils import run_bass_kernel_spmd

F16 = mybir.dt.float16
F32 = mybir.dt.float32

B, C, T, O = 8, 16, 131072, 16
P = 128
CH = 512                  # time elems per partition row per supertile
NST = T // (P * CH)       # 2 supertiles
V = 8                     # sub-block length
NU = 8                    # sub-blocks per window
S = NU * V                # 64
NW = CH // S              # windows per partition row = 8

WARMUP_MMS = 12

_cache = {}


def _build_nc(first_u):
    DU = NU - first_u          # active sub-blocks (trailing)
    NPRE = DU - 1              # blocks contributing prefix totals
    KEEP = DU * V              # stored positions per window (s >= fill)
    NWIN = NST * NW            # 16 windows per partition row overall
    XW = DU * P                # xT cols per window
    YW = O * KEEP              # y cols per window, (o, s)-major
    PREB = DU * 128            # intra cols per window
    NP = NWIN // 2             # window pairs
    PAIR = 2 * YW
    # packed pre widths: first covers all DU targets (zero block for du=0)
    pre_ln = [DU * O] + [(DU - pu - 1) * O for pu in range(1, NPRE)]
    WPC = sum(pre_ln) if NPRE else 0

    nc = bacc.Bacc("TRN2", target_bir_lowering=False, debug=False)
    xt_d = nc.dram_tensor("xT", (P, NWIN * XW), F16, kind="ExternalInput")
    wc_d = nc.dram_tensor("wc", (P, DU * 128 + max(WPC, 1)), F16,
                          kind="ExternalInput")
    y_d = nc.dram_tensor("y", (P, NWIN * YW), F16, kind="ExternalOutput")

    with tile.TileContext(nc) as tc:
        with (
            tc.tile_pool(name="const", bufs=1) as cp,
            tc.tile_pool(name="xtp", bufs=1) as xtp,
            tc.tile_pool(name="outp", bufs=4) as outp,
            tc.tile_pool(name="mid", bufs=4) as mid,
            tc.tile_pool(name="psW", bufs=2, space="PSUM") as psW,
        ):
            # loads: pair0 chunk first on sync; weights first on scalar;
            # later pairs in 2-pair chunks
            xt = xtp.tile([P, NWIN * XW], F16, name="xt")
            nc.sync.dma_start(xt[:, 0:2 * XW], xt_d.ap()[:, 0:2 * XW])
            wc = cp.tile([P, DU * 128 + max(WPC, 1)], F16, name="wc")
            nc.scalar.dma_start(wc[:], wc_d.ap())
            nc.scalar.dma_start(xt[:, 2 * XW:4 * XW],
                                xt_d.ap()[:, 2 * XW:4 * XW])
            nc.sync.dma_start(xt[:, 4 * XW:8 * XW],
                              xt_d.ap()[:, 4 * XW:8 * XW])
            nc.scalar.dma_start(xt[:, 8 * XW:12 * XW],
                                xt_d.ap()[:, 8 * XW:12 * XW])
            nc.sync.dma_start(xt[:, 12 * XW:16 * XW],
                              xt_d.ap()[:, 12 * XW:16 * XW])
            w_intra = wc[:, 0:DU * 128]
            w_pre = wc[:, DU * 128:DU * 128 + max(WPC, 1)]

            # PE warm-up on a memset tile: keeps the HAM activity window
            # busy (PE at 2.4GHz) until pair 0's data lands
            warm_sb = cp.tile([P, 512], F16, name="warm_sb")
            nc.gpsimd.memset(warm_sb[:], 0)
            warm_pw = psW.tile([P, 512], F32, name="warm_pw", tag="pw")
            for _ in range(WARMUP_MMS):
                nc.tensor.matmul(
                    warm_pw[:, 0:512], warm_sb[:, 0:128], warm_sb[:, 0:512],
                    start=True, stop=True, skip_group_check=True,
                )

            for pi in range(NP):
                pw = psW.tile([P, 2 * PREB + 256], F32, name="pw", tag="pw")
                ot = outp.tile([P, PAIR], F16, name="ot", tag="ot")
                for h in range(2):
                    wi = 2 * pi + h
                    ib = h * PREB
                    pb = 2 * PREB + h * 128
                    off = 0
                    for pu in range(DU):
                        xb = xt[:, wi * XW + pu * 128:wi * XW + (pu + 1) * 128]
                        nc.tensor.matmul(
                            pw[:, ib + pu * 128:ib + (pu + 1) * 128],
                            xb, w_intra[:, pu * 128:(pu + 1) * 128],
                            start=True, stop=True, skip_group_check=True,
                        )
                        if pu < NPRE:
                            ln = pre_ln[pu]
                            nc.tensor.matmul(
                                pw[:, pb + 128 - ln:pb + 128],
                                xb, w_pre[:, off:off + ln],
                                start=(pu == 0), stop=(pu == NPRE - 1),
                                skip_group_check=True,
                            )
                            off += ln
                    # prefix totals to SBUF on ACT (TT allows only one PSUM
                    # input); then eviction: out[(o,du,v)] = intra + prefix
                    pre_sb = mid.tile([P, 128], F32, name="pre_sb", tag="pre")
                    nc.scalar.copy(pre_sb[:], pw[:, pb:pb + 128])
                    out3 = ot[:, h * YW:(h + 1) * YW].rearrange(
                        "p (o du v) -> p o du v", o=O, du=DU, v=V)
                    in1 = pw[:, ib:ib + PREB].rearrange(
                        "p (du v o) -> p o du v", du=DU, v=V, o=O)
                    in2 = pre_sb[:].rearrange(
                        "p (u o) -> p o u", u=NU
                    )[:, :, first_u:].unsqueeze(3).broadcast_to([P, O, DU, V])
                    nc.vector.tensor_add(out3, in1, in2)
                eng = nc.scalar if pi % 2 == 0 else nc.sync
                eng.dma_start(
                    y_d.ap()[:, pi * PAIR:(pi + 1) * PAIR], ot[:])
    nc.compile()
    return nc


def _host_constants(weight, bias, n_discard, n_keep):
    Swin = n_discard + n_keep
    assert Swin == S and n_discard % V == 0
    first_u = n_discard // V
    DU = NU - first_u
    NPRE = DU - 1

    w = weight.reshape(O, C, n_keep).transpose(2, 1, 0).astype(np.float32)
    w_full = np.concatenate(
        [np.zeros((n_discard, C, O), np.float32), w], axis=0
    )  # (S, C, O)

    # triangular intra: w_intra[(c,vp), du*128 + v*16 + o] = blk[du,vp,c,o], vp<=v
    blk = np.stack([w_full[(first_u + du) * V:(first_u + du + 1) * V]
                    for du in range(DU)])          # (DU, V, C, O)
    tri = np.zeros((DU, C, V, V, O), np.float32)   # (du, c, vp, v, o)
    vp = np.arange(V)
    for v in range(V):
        tri[:, :, vp <= v, v, :] = blk.transpose(0, 2, 1, 3)[:, :, vp <= v]
    w_intra = tri.reshape(DU, C * V, V * O).transpose(1, 0, 2).reshape(
        P, DU * 128)

    # packed w_pre: source pu's block-total weights replicated per later
    # target; first matmul covers all DU targets with a zero du=0 block
    cols = []
    for pu in range(NPRE):
        rep = NPRE - pu
        tot = blk[pu].transpose(1, 0, 2).reshape(C * V, O)
        if pu == 0:
            cols.append(np.zeros((C * V, O), np.float32))
        cols.append(np.tile(tot, (1, rep)))
    w_pre = (np.concatenate(cols, axis=1) if cols
             else np.zeros((C * V, 1), np.float32))
    wc = np.concatenate([w_intra, w_pre], axis=1)

    consts = {"wc": np.ascontiguousarray(wc).astype(np.float16)}
    return consts, first_u


def _run(inputs, trace=False):
    x = np.asarray(inputs["x"], dtype=np.float32)
    weight = np.asarray(inputs["weight"], dtype=np.float32)
    bias = np.asarray(inputs["bias"], dtype=np.float32)
    n_discard = int(inputs["n_discard"])
    n_keep = int(inputs["n_keep"])
    assert x.shape == (B, C, T) and weight.shape == (O, C * n_keep)

    consts, first_u = _host_constants(weight, bias, n_discard, n_keep)
    DU = NU - first_u
    key = ("nc", first_u)
    if key not in _cache:
        _cache[key] = _build_nc(first_u)
    nc = _cache[key]

    # marshal x: fp16, drop s < n_discard, pre-transpose to
    # [(c,v), (st, w, u, tchunk p)]
    x16 = x.astype(np.float16)
    xr = x16.reshape(B, C, NST, P, NW, NU, V)[:, :, :, :, :, first_u:, :]
    xT = np.ascontiguousarray(xr.transpose(0, 1, 6, 2, 4, 5, 3)).reshape(
        B, C * V, NST * NW * DU * P)

    in_maps = []
    for b in range(B):
        m = dict(consts)
        m["xT"] = xT[b]
        in_maps.append(m)
    res = run_bass_kernel_spmd(nc, in_maps, list(range(B)), trace=trace)

    # assemble: device gives (window, o, s)-major cumsum for s >= n_discard;
    # host adds bias and fills discard positions
    KEEP = DU * V
    y = np.empty((B, O, T), np.float32)
    yw = y.reshape(B, O, NST, P, NW, S)
    yw[:, :, :, :, :, :n_discard] = bias.astype(np.float32)[
        None, :, None, None, None, None]
    for b in range(B):
        yd = res.results[b]["y"].reshape(P, NST, NW, O, KEEP)
        arr = yd.transpose(3, 1, 0, 2, 4).astype(np.float32)
        arr += bias[:, None, None, None, None]
        yw[b, :, :, :, :, n_discard:] = arr
    return y, res


def kernel(**inputs):
    y, _ = _run(inputs, trace=False)
    return y


# revision 10
# speedup vs baseline: 1.0119x; 1.0119x over previous
"""Trainium2 Bass kernel for nn_CumulativeFlattenedLinear (segment_reduce).

Per window of S=64 timesteps: per-timestep C->O projection (weights zero for
the first n_discard steps) + causal cumsum within the window, plus bias.

Strategy (data-parallel over batch, 1 batch element per core):
  - Host marshals x to fp16, drops the n_discard zero-weight positions of
    every window (25% of the data), pre-transposed as x^T[(c,v'), ...] so the
    device reads matmul stationary slabs directly.
  - Per window: DU triangular "intra" matmuls produce within-block causal
    cumsums in PSUM; packed "pre" matmuls accumulate block-prefix totals into
    a 128-col pre region in its own PSUM bank (first pre matmul covers all
    DU targets, with a zero block so du=0 reads exact 0).  Intra and pre
    matmuls are interleaved so each x-block's stationary weights are loaded
    once.
  - Eviction: one DVE tensor-tensor ADD per window reads intra + broadcast
    pre straight from PSUM and writes (o, s)-major fp16 output tiles.
  - Bias is applied host-side (discard positions are bias-only, also filled
    host-side), so the device does no bias work.
  - PE warm-up matmuls keep the HAM clock busy until pair 0's data lands;
    loads are ordered so pair 0's chunk is first on the wire.
"""
import numpy as np

import concourse.bass as bass
import concourse.tile as tile
from concourse import bacc, mybir
from concourse.bass_utils import run_bass_kernel_spmd

F16 = mybir.dt.float16
F32 = mybir.dt.float32

B, C, T, O = 8, 16, 131072, 16
P = 128
CH = 512                  # time elems per partition row per supertile
NST = T // (P * CH)       # 2 supertiles
V = 8                     # sub-block length
NU = 8                    # sub-blocks per window
S = NU * V                # 64
NW = CH // S              # windows per partition row = 8

WARMUP_MMS = 12

_cache = {}


def _build_nc(first_u):
    DU = NU - first_u          # active sub-blocks (trailing)
    NPRE = DU - 1              # blocks contributing prefix totals
    KEEP = DU * V              # stored positions per window (s >= fill)
    NWIN = NST * NW            # 16 windows per partition row overall
    XW = DU * P                # xT cols per window
    YW = O * KEEP              # y cols per window, (o, s)-major
    PREB = DU * 128            # intra cols per window
    NP = NWIN // 2             # window pairs
    PAIR = 2 * YW
    # packed pre widths: first covers all DU targets (zero block for du=0)
    pre_ln = [DU * O] + [(DU - pu - 1) * O for pu in range(1, NPRE)]
    WPC = sum(pre_ln) if NPRE else 0

    nc = bacc.Bacc("TRN2", target_bir_lowering=False, debug=False)
    xt_d = nc.dram_tensor("xT", (P, NWIN * XW), F16, kind="ExternalInput")
    wc_d = nc.dram_tensor("wc", (P, DU * 128 + max(WPC, 1)), F16,
                          kind="ExternalInput")
    y_d = nc.dram_tensor("y", (P, NWIN * YW), F16, kind="ExternalOutput")

    with tile.TileContext(nc) as tc:
        with (
            tc.tile_pool(name="const", bufs=1) as cp,
            tc.tile_pool(name="xtp", bufs=1) as xtp,
            tc.tile_pool(name="outp", bufs=4) as outp,
            tc.tile_pool(name="mid", bufs=4) as mid,
            tc.tile_pool(name="psW", bufs=2, space="PSUM") as psW,
        ):
            # loads: pair0 chunk first on sync; weights first on scalar;
            # later pairs in 2-pair chunks
            xt = xtp.tile([P, NWIN * XW], F16, name="xt")
            nc.sync.dma_start(xt[:, 0:2 * XW], xt_d.ap()[:, 0:2 * XW])
            wc = cp.tile([P, DU * 128 + max(WPC, 1)], F16, name="wc")
            nc.scalar.dma_start(wc[:], wc_d.ap())
            nc.scalar.dma_start(xt[:, 2 * XW:4 * XW],
                                xt_d.ap()[:, 2 * XW:4 * XW])
            nc.sync.dma_start(xt[:, 4 * XW:8 * XW],
                              xt_d.ap()[:, 4 * XW:8 * XW])
            nc.scalar.dma_start(xt[:, 8 * XW:12 * XW],
                                xt_d.ap()[:, 8 * XW:12 * XW])
            nc.sync.dma_start(xt[:, 12 * XW:16 * XW],
                              xt_d.ap()[:, 12 * XW:16 * XW])
            w_intra = wc[:, 0:DU * 128]
            w_pre = wc[:, DU * 128:DU * 128 + max(WPC, 1)]

            # PE warm-up on a memset tile: keeps the HAM activity window
            # busy (PE at 2.4GHz) until pair 0's data lands
            warm_sb = cp.tile([P, 512], F16, name="warm_sb")
            nc.gpsimd.memset(warm_sb[:], 0)
            warm_pw = psW.tile([P, 512], F32, name="warm_pw", tag="pw")
            for _ in range(WARMUP_MMS):
                nc.tensor.matmul(
                    warm_pw[:, 0:512], warm_sb[:, 0:128], warm_sb[:, 0:512],
                    start=True, stop=True, skip_group_check=True,
                )

            # pair psum layout: each window owns 2 whole banks (1024 f32):
            # [intra 768 | pre 128 | pad 128] so windows/pairs never share a
            # bank (matmul start=True invalidates has_written bank-wide, so
            # bank sharing serializes readers against later matmuls)
            for pi in range(NP):
                pw = psW.tile([P, 2048], F32, name="pw", tag="pw")
                ot = outp.tile([P, PAIR], F16, name="ot", tag="ot")
                for h in range(2):
                    wi = 2 * pi + h
                    ib = h * 1024
                    pb = h * 1024 + PREB
                    # all intra first, then the pre chain: a start=True
                    # matmul clears has_written bank-wide, so no intra
                    # matmul may land between pre-chain accumulations
                    for pu in range(DU):
                        nc.tensor.matmul(
                            pw[:, ib + pu * 128:ib + (pu + 1) * 128],
                            xt[:, wi * XW + pu * 128:wi * XW + (pu + 1) * 128],
                            w_intra[:, pu * 128:(pu + 1) * 128],
                            start=True, stop=True, skip_group_check=True,
                        )
                    off = 0
                    for pu in range(NPRE):
                        ln = pre_ln[pu]
                        nc.tensor.matmul(
                            pw[:, pb + 128 - ln:pb + 128],
                            xt[:, wi * XW + pu * 128:wi * XW + (pu + 1) * 128],
                            w_pre[:, off:off + ln],
                            start=(pu == 0), stop=(pu == NPRE - 1),
                            skip_group_check=True,
                        )
                        off += ln
                    # prefix totals to SBUF on ACT (TT allows only one PSUM
                    # input); then eviction: out[(o,du,v)] = intra + prefix
                    pre_sb = mid.tile([P, 128], F32, name="pre_sb", tag="pre")
                    nc.scalar.copy(pre_sb[:], pw[:, pb:pb + 128])
                    out3 = ot[:, h * YW:(h + 1) * YW].rearrange(
                        "p (o du v) -> p o du v", o=O, du=DU, v=V)
                    in1 = pw[:, ib:ib + PREB].rearrange(
                        "p (du v o) -> p o du v", du=DU, v=V, o=O)
                    in2 = pre_sb[:].rearrange(
                        "p (u o) -> p o u", u=NU
                    )[:, :, first_u:].unsqueeze(3).broadcast_to([P, O, DU, V])
                    nc.vector.tensor_add(out3, in1, in2)
                eng = nc.scalar if pi % 2 == 0 else nc.sync
                eng.dma_start(
                    y_d.ap()[:, pi * PAIR:(pi + 1) * PAIR], ot[:])
    nc.compile()
    return nc


def _host_constants(weight, bias, n_discard, n_keep):
    Swin = n_discard + n_keep
    assert Swin == S and n_discard % V == 0
    first_u = n_discard // V
    DU = NU - first_u
    NPRE = DU - 1

    w = weight.reshape(O, C, n_keep).transpose(2, 1, 0).astype(np.float32)
    w_full = np.concatenate(
        [np.zeros((n_discard, C, O), np.float32), w], axis=0
    )  # (S, C, O)

    # triangular intra: w_intra[(c,vp), du*128 + v*16 + o] = blk[du,vp,c,o], vp<=v
    blk = np.stack([w_full[(first_u + du) * V:(first_u + du + 1) * V]
                    for du in range(DU)])          # (DU, V, C, O)
    tri = np.zeros((DU, C, V, V, O), np.float32)   # (du, c, vp, v, o)
    vp = np.arange(V)
    for v in range(V):
        tri[:, :, vp <= v, v, :] = blk.transpose(0, 2, 1, 3)[:, :, vp <= v]
    w_intra = tri.reshape(DU, C * V, V * O).transpose(1, 0, 2).reshape(
        P, DU * 128)

    # packed w_pre: source pu's block-total weights replicated per later
    # target; first matmul covers all DU targets with a zero du=0 block
    cols = []
    for pu in range(NPRE):
        rep = NPRE - pu
        tot = blk[pu].transpose(1, 0, 2).reshape(C * V, O)
        if pu == 0:
            cols.append(np.zeros((C * V, O), np.float32))
        cols.append(np.tile(tot, (1, rep)))
    w_pre = (np.concatenate(cols, axis=1) if cols
             else np.zeros((C * V, 1), np.float32))
    wc = np.concatenate([w_intra, w_pre], axis=1)

    consts = {"wc": np.ascontiguousarray(wc).astype(np.float16)}
    return consts, first_u


def _run(inputs, trace=False):
    x = np.asarray(inputs["x"], dtype=np.float32)
    weight = np.asarray(inputs["weight"], dtype=np.float32)
    bias = np.asarray(inputs["bias"], dtype=np.float32)
    n_discard = int(inputs["n_discard"])
    n_keep = int(inputs["n_keep"])
    assert x.shape == (B, C, T) and weight.shape == (O, C * n_keep)

    consts, first_u = _host_constants(weight, bias, n_discard, n_keep)
    DU = NU - first_u
    key = ("nc", first_u)
    if key not in _cache:
        _cache[key] = _build_nc(first_u)
    nc = _cache[key]

    # marshal x: fp16, drop s < n_discard, pre-transpose to
    # [(c,v), (st, w, u, tchunk p)]
    x16 = x.astype(np.float16)
    xr = x16.reshape(B, C, NST, P, NW, NU, V)[:, :, :, :, :, first_u:, :]
    xT = np.ascontiguousarray(xr.transpose(0, 1, 6, 2, 4, 5, 3)).reshape(
        B, C * V, NST * NW * DU * P)

    in_maps = []
    for b in range(B):
        m = dict(consts)
        m["xT"] = xT[b]
        in_maps.append(m)
    res = run_bass_kernel_spmd(nc, in_maps, list(range(B)), trace=trace)

    # assemble: device gives (window, o, s)-major cumsum for s >= n_discard;
    # host adds bias and fills discard positions
    KEEP = DU * V
    y = np.empty((B, O, T), np.float32)
    yw = y.reshape(B, O, NST, P, NW, S)
    yw[:, :, :, :, :, :n_discard] = bias.astype(np.float32)[
        None, :, None, None, None, None]
    for b in range(B):
        yd = res.results[b]["y"].reshape(P, NST, NW, O, KEEP)
        arr = yd.transpose(3, 1, 0, 2, 4).astype(np.float32)
        arr += bias[:, None, None, None, None]
        yw[b, :, :, :, :, n_discard:] = arr
    return y, res


def kernel(**inputs):
    y, _ = _run(inputs, trace=False)
    return y
